# revision 16
# baseline (speedup 1.0000x reference)
"""Trainium2 Bass kernel for nn_AttentionMemory (sparse_attention).

Reference computation (per batch b):
    mk = Mk[b].reshape(CK, N); qk = Qk[b].reshape(CK, N)
    affinity[m, q] = softmax_m( (2*mk[:,m]@qk[:,q] - |mk[:,m]|^2) / sqrt(CK) )

Sharding: 8 cores = 4 batches x 2 query-halves. Each core computes the full
memory (softmax) axis for 2048 of one batch's queries — no collectives.

Per-core layout: queries on partitions (16 q-tiles of 128), memory positions
on the free axis. Per q-tile, the 4096-wide pre-softmax row is built in two
2048-column halves, each a 4-bank PSUM tile (double-buffered ping-pong):
    for j in 0..4: psum[:, j*512:] = matmul(qk_tile, mk)        (start)
    for j in 0..4: psum[:, j*512:] += matmul(-0.5*ones, mksq)   (stop)
ScalarE applies exp(2/sqrt(CK)*psum) over the whole 2048-wide half in ONE
activation (PSUM reads may span banks; only matmul writes are bank-limited),
writing bf16 numerators with a fused fp32 row-sum (accum_out). VectorE adds
the 2 partials, takes the reciprocal and applies it as a per-partition
tensor_scalar multiply at 4x bf16 rate; DMA writes bf16 [128, 2048] blocks.
The host transposes [q, m] -> [m, q] and casts bf16 -> fp32 while gathering.

Implementation notes:
  * Matmul operands are fp16: measured HW runs fp32r matmuls at ~2 cycles/
    column while fp16/bf16 stream at 1 cycle/column. fp16 keeps 10 mantissa
    bits: end-to-end rel err ~3e-3 (numpy-simulated) vs the 2e-2 gate. PSUM
    accumulation stays fp32.
  * The "-0.5*|mk|^2" rank-1 term intentionally stays a full K=128 matmul
    stream: a cheaper 4x row-tiled K=1 formulation was measured SLOWER
    because the PE then idles ~40% and the HAM clock gate re-throttles it to
    1.2 GHz for the whole run (matmuls 427ns instead of 216ns) plus ~400ns
    tiling-mode-switch drains. The dense 2-stream version keeps the PE ~98%
    busy and warm, balancing ScalarE's exp chain almost exactly.
  * Within a half, the 4 "qk@mk" matmuls share the q-tile stationary and the
    4 "-0.5*|mk|^2" matmuls share the ones stationary; the
    _elide_redundant_ldweights BIR pass drops the repeated LDWEIGHTS (bass
    emits one per matmul). With one load per group the same-weight matmuls
    stream back-to-back at 216ns/512-col (per-matmul reloads forced a full
    drain/fill serialization: 379ns + 117ns gap each).
  * The host uploads the inputs pre-cast to fp16 (the kernel only ever
    consumes fp16 operands), halving input DMA bytes; chunked transfers are
    split across both HWDGE rings (sync + scalar) so they drain in
    parallel, and mksq comes from fp16*fp16 on VectorE at 2x rate. ~48
    dummy 128-column matmuls warm the HAM clock gate while the DMAs land
    so the first real bursts stream at 2.4 GHz.
  * bf16 numerators halve DVE normalize time (4x mode) and halve the output
    DMA bytes (16.8MB/core); the logits are bounded (~[-30, +8]) so no
    max-subtraction pass is needed. The normalize + store is split per
    2048-column half so the final store starts one tensor_scalar earlier.
  * Walrus caps instructions at one sync wait. After Tile scheduling, any
    instruction with N>1 waits has N-1 of them spilled onto single-wait Drain
    instructions inserted before it on the same engine — semantically
    equivalent (waits are an AND over monotonic semaphores, executed in order
    on the same sequencer).
"""
import math
import numpy as np

import bass_rust
from concourse import bass, tile, mybir
from concourse.bass_utils import run_bass_kernel_spmd

B, CK, HH, WW = 4, 128, 64, 64
N = HH * WW            # 4096 memory positions / queries per batch
QH = N // 2            # 2048 queries per core
N_CORES = 8
QTILE = 128            # queries per q-tile (PSUM partition dim)
MCHUNK = 512           # memory cols per matmul (one PSUM bank, fp32)
MHALF = 2048           # memory cols per PSUM tile / activation
ICHUNK = 1024          # input DMA chunk width
SCALE = 2.0 / math.sqrt(CK)
F32 = mybir.dt.float32
F16 = mybir.dt.float16
BF16 = mybir.dt.bfloat16


def _elide_redundant_ldweights(nc):
    """Remove InstLdweights that reload the weights already resident in the
    PE array (same tensor, offset, access pattern, dtype and mode as the
    previous load, with only matmuls in between). A redundant load that
    carries semaphore waits/updates is converted to a Drain with identical
    sync_info so the synchronization graph is unchanged; one with no sync
    info is dropped. Standalone-LDWEIGHTS + non-self-loading matmul is only
    correct for 16-bit weights (fp32/fp32r break in walrus) — all matmuls
    here are fp16.
    """
    def key(ins):
        w = ins.ins[0]
        return (
            getattr(w, "memref", None), getattr(w, "offset", None),
            str(getattr(w, "ap", None)), str(getattr(w, "dtype", None)),
            str(getattr(ins, "perf_mode", None)),
            str(getattr(ins, "is_transpose", None)),
            str(getattr(ins, "tile_position", None)),
        )

    for fn in nc.m.functions:
        for blk in fn.blocks:
            last_key = None
            new_il = []
            for ins in blk.instructions:
                tn = type(ins).__name__
                if tn == "InstLdweights":
                    k = key(ins)
                    if k == last_key:
                        si = getattr(ins, "sync_info", None)
                        if si is not None and (si.on_wait or si.on_update):
                            d = mybir.InstDrain(
                                name=f"{ins.name}_ldwskip",
                                ins=[], outs=[], bass_is_fusable=False)
                            d.engine = ins.engine
                            d.sync_info = si
                            new_il.append(d)
                        continue
                    last_key = k
                elif tn == "InstMatmult":
                    pass  # matmuls leave the loaded weights untouched
                elif getattr(ins, "engine", None) == getattr(
                        nc.tensor, "engine", None):
                    # any other PE-queue instruction: be conservative
                    last_key = None
                new_il.append(ins)
            blk.instructions = new_il


def _build():
    nc = bass.Bass("TRN2", target_bir_lowering=False, debug=False,
                   num_devices=N_CORES)
    mk_d = nc.dram_tensor("mk", [CK, N], F16, kind="ExternalInput").ap()
    qk_d = nc.dram_tensor("qk", [CK, QH], F16, kind="ExternalInput").ap()
    out_d = nc.dram_tensor("out", [QH, N], BF16, kind="ExternalOutput").ap()

    n_qt = QH // QTILE          # 16
    n_mc = N // ICHUNK          # 4 mk chunks
    n_qc = QH // ICHUNK         # 2 qk chunks
    with tile.TileContext(nc) as tc:
        with tc.tile_pool(name="inp", bufs=1) as inp_pool, \
             tc.tile_pool(name="exp", bufs=2) as exp_pool, \
             tc.tile_pool(name="outb", bufs=4) as out_pool, \
             tc.tile_pool(name="small", bufs=4) as small_pool, \
             tc.tile_pool(name="psum", bufs=2, space="PSUM") as psum_pool:

            mk_h = [inp_pool.tile([CK, ICHUNK], F16, name=f"mkh{c}", tag=f"mkh{c}")
                    for c in range(n_mc)]
            qk_h = [inp_pool.tile([CK, ICHUNK], F16, name=f"qkh{c}", tag=f"qkh{c}")
                    for c in range(n_qc)]
            mksq_h = [inp_pool.tile([CK, ICHUNK], F16, name=f"mksqh{c}", tag=f"mksqh{c}")
                      for c in range(n_mc)]
            mhalf = inp_pool.tile([128, 128], F16, tag="mhalf")
            qk_tiles = [inp_pool.tile([CK, QTILE], F16, name=f"qkt{t}", tag=f"qkt{t}")
                        for t in range(n_qt)]

            # fp16 input DMAs split across both HWDGE rings (sync=SP ring,
            # scalar=ACT ring) so the transfers drain in parallel,
            # first-needed chunks first (a SWDGE third path was measured
            # slower to land than the second slot of a HWDGE ring).
            nc.sync.dma_start(out=mk_h[0][:], in_=mk_d[:, 0:ICHUNK])
            nc.scalar.dma_start(out=qk_h[0][:], in_=qk_d[:, 0:ICHUNK])
            nc.scalar.dma_start(out=mk_h[1][:], in_=mk_d[:, ICHUNK:2 * ICHUNK])
            nc.sync.dma_start(out=mk_h[2][:], in_=mk_d[:, 2 * ICHUNK:3 * ICHUNK])
            nc.scalar.dma_start(out=mk_h[3][:], in_=mk_d[:, 3 * ICHUNK:N])
            nc.sync.dma_start(out=qk_h[1][:], in_=qk_d[:, ICHUNK:QH])

            nc.vector.memset(mhalf[:], -0.5)

            # HAM warm-up: the PE clock gate only reaches 2.4 GHz after
            # ~3.4us of sustained matmul activity. The input DMAs take
            # ~6us to land, so burn the wait on cheap 128-column dummy
            # matmuls (the first q-tile's start=True overwrites the
            # garbage); without this the first real bursts run at 1.2 GHz.
            ps_warm = psum_pool.tile([QTILE, MHALF], F32, tag="ps")
            for w in range(48):
                nc.tensor.matmul(ps_warm[:, 0:128], mhalf[:],
                                 mhalf[:, 0:128], start=True, stop=True)

            nc.vector.tensor_mul(mksq_h[0][:], mk_h[0][:], mk_h[0][:])
            nc.vector.tensor_mul(mksq_h[1][:], mk_h[1][:], mk_h[1][:])
            for t in range(8):
                nc.vector.tensor_copy(
                    qk_tiles[t][:], qk_h[0][:, t * QTILE:(t + 1) * QTILE])
            nc.vector.tensor_mul(mksq_h[2][:], mk_h[2][:], mk_h[2][:])
            nc.vector.tensor_mul(mksq_h[3][:], mk_h[3][:], mk_h[3][:])
            for t in range(8, n_qt):
                off = t * QTILE - ICHUNK
                nc.vector.tensor_copy(
                    qk_tiles[t][:], qk_h[1][:, off:off + QTILE])

            for t in range(n_qt):
                exp_t = exp_pool.tile([QTILE, N], BF16, tag="exp")
                parts = small_pool.tile([QTILE, 2], F32, tag="parts")
                for h in range(2):
                    ps = psum_pool.tile([QTILE, MHALF], F32, tag="ps")
                    for j in range(4):
                        g = h * MHALF + j * MCHUNK
                        c, off = divmod(g, ICHUNK)
                        nc.tensor.matmul(
                            ps[:, j * MCHUNK:(j + 1) * MCHUNK], qk_tiles[t],
                            mk_h[c][:, off:off + MCHUNK],
                            start=True, stop=False)
                    for j in range(4):
                        g = h * MHALF + j * MCHUNK
                        c, off = divmod(g, ICHUNK)
                        nc.tensor.matmul(
                            ps[:, j * MCHUNK:(j + 1) * MCHUNK], mhalf[:],
                            mksq_h[c][:, off:off + MCHUNK],
                            start=False, stop=True)
                    nc.scalar.activation(
                        exp_t[:, h * MHALF:(h + 1) * MHALF], ps[:],
                        mybir.ActivationFunctionType.Exp,
                        scale=SCALE, accum_out=parts[:, h:h + 1])
                # denominator: add the 2 partials, then reciprocal
                s1 = small_pool.tile([QTILE, 1], F32, tag="s1")
                rec_t = small_pool.tile([QTILE, 1], F32, tag="rec")
                nc.vector.tensor_add(s1[:], parts[:, 0:1], parts[:, 1:2])
                nc.vector.reciprocal(rec_t[:], s1[:])
                for h in range(2):
                    o = out_pool.tile([QTILE, MHALF], BF16, tag="o")
                    nc.vector.tensor_scalar_mul(
                        o[:], exp_t[:, h * MHALF:(h + 1) * MHALF], rec_t[:])
                    # last tile: put one store on the (by then idle) ACT
                    # ring so the two final completion receipts overlap;
                    # earlier tiles stay off the ACT ring.
                    eng = nc.scalar if (t == n_qt - 1 and h == 0) else nc.sync
                    eng.dma_start(
                        out=out_d[t * QTILE:(t + 1) * QTILE,
                                  h * MHALF:(h + 1) * MHALF],
                        in_=o[:])
    _elide_redundant_ldweights(nc)
    _strip_self_waits(nc)
    return nc


def _strip_self_waits(nc):
    """Walrus rejects instructions carrying more than one sync wait.

    Conservative fix: for any instruction with N>1 waits, keep the last wait
    on the instruction and spill the other N-1 onto single-wait Drain
    instructions inserted immediately before it on the same engine. All waits
    still execute, in program order, on the same sequencer; semaphores are
    monotonic so splitting an AND of waits into a sequence is equivalent.
    """
    for fn in nc.m.functions:
        for blk in fn.blocks:
            il = blk.instructions
            new_il = []
            changed = False
            for ins in il:
                si = getattr(ins, "sync_info", None)
                if si is not None and len(si.on_wait) > 1:
                    changed = True
                    waits = list(si.on_wait)
                    for k, w in enumerate(waits[:-1]):
                        d = mybir.InstDrain(
                            name=f"{ins.name}_w{k}",
                            ins=[], outs=[], bass_is_fusable=False)
                        d.engine = ins.engine
                        d.sync_info = bass_rust.SyncInfo(on_wait=[w],
                                                         on_update=[])
                        new_il.append(d)
                    ins.sync_info = bass_rust.SyncInfo(on_wait=[waits[-1]],
                                                      on_update=si.on_update)
                new_il.append(ins)
            if changed:
                blk.instructions = new_il


_NC_CACHE = None


def kernel(Mk: np.ndarray, Qk: np.ndarray) -> np.ndarray:
    global _NC_CACHE
    if _NC_CACHE is None:
        _NC_CACHE = _build()
    nc = _NC_CACHE

    Mk = np.asarray(Mk, dtype=np.float32)
    Qk = np.asarray(Qk, dtype=np.float32)

    in_maps = []
    for c in range(N_CORES):
        b, half = c // 2, c % 2
        mk = np.ascontiguousarray(Mk[b].reshape(CK, N).astype(np.float16))
        qk = np.ascontiguousarray(
            Qk[b].reshape(CK, N)[:, half * QH:(half + 1) * QH].astype(np.float16))
        in_maps.append({"mk": mk, "qk": qk})

    res = run_bass_kernel_spmd(nc, in_maps, core_ids=list(range(N_CORES)))

    out = np.empty((B, N, N), dtype=np.float32)
    for c in range(N_CORES):
        b, half = c // 2, c % 2
        out[b, :, half * QH:(half + 1) * QH] = \
            np.asarray(res.results[c]["out"]).astype(np.float32).T
    return out
